# revision 1
# baseline (speedup 1.0000x reference)
"""HSpatialHyperGCN Trainium2 kernel.

Shapes (hardcoded): x (4, 64, 64, 64); N = 4096 nodes per batch; 4 heads x 64
inter channels; top-5 cosine-similarity hypergraph; uniform degree 6 Laplacian;
hydra attention (global kv); 1x1-conv + folded-BN chain.

Sharding: 8 cores = 4 batches x 2 node-halves. Each core:
  - builds the full normalized k|v node-major table (4096, 512) in DRAM
  - computes sim rows for its 2048 nodes against all 4096 (PE), top-8 via
    DVE max/max_index, takes top-5 indices
  - gathers 5 neighbor rows per node (indirect DMA), forms 6*(kL|vL),
    accumulates sum_n kL*vL partially
  - AllReduce (pairs) of the 256-float kv, folds kv into wp, runs the
    conv->BN->relu chain on its 2048 columns.

Key algebraic facts exploited: node degree is uniformly 6 (every node emits
exactly K=5 edges), so the Laplacian is (f[n] + sum_{j in top5(n)} f[j]) / 6
and top1 is always the node itself; kv folds into wp; BN folds into conv
weights; bp folds through w1.
"""

import sys

sys.path.insert(0, "/opt/trn_rl_repo")

import numpy as np

from concourse import bass, mybir, tile, bacc
from concourse.bass_utils import run_bass_kernel_spmd

F32 = mybir.dt.float32
U32 = mybir.dt.uint32
AF = mybir.ActivationFunctionType
ALU = mybir.AluOpType

B, C, H, W = 4, 64, 64, 64
N = H * W            # 4096
NH = 4               # heads
INTER = 64
OC = NH * INTER      # 256
K = 5
ROWS = N // 2        # 2048 rows per core
BN_EPS = 1e-5

_CACHE = {}


def _build_bass(collective=True):
    nc = bacc.Bacc(None, target_bir_lowering=False, debug=False, num_devices=8)

    # per-core external inputs
    xa = nc.dram_tensor("xa", [C + 1, N], F32, kind="ExternalInput")       # x[b] + ones row
    xr = nc.dram_tensor("xr", [C + 1, ROWS], F32, kind="ExternalInput")    # own row half
    wkv = nc.dram_tensor("wkv", [C + 1, 2 * OC], F32, kind="ExternalInput")
    wq = nc.dram_tensor("wq", [C + 1, OC], F32, kind="ExternalInput")
    wpt = nc.dram_tensor("wpt", [128, 2, 64], F32, kind="ExternalInput")
    w1t = nc.dram_tensor("w1t", [64, 64], F32, kind="ExternalInput")
    w2t = nc.dram_tensor("w2t", [64, 64], F32, kind="ExternalInput")
    b1ff = nc.dram_tensor("b1ff", [64, 1], F32, kind="ExternalInput")
    b2f = nc.dram_tensor("b2f", [64, 1], F32, kind="ExternalInput")
    bo1 = nc.dram_tensor("bo1", [128, 2], F32, kind="ExternalInput")
    bo2 = nc.dram_tensor("bo2", [2, 128], F32, kind="ExternalInput")
    ones64 = nc.dram_tensor("ones64", [64, 1], F32, kind="ExternalInput")
    one1_64 = nc.dram_tensor("one1_64", [1, 64], F32, kind="ExternalInput")
    ones128 = nc.dram_tensor("ones128", [128, 1], F32, kind="ExternalInput")

    out_half = nc.dram_tensor("out_half", [64, ROWS], F32, kind="ExternalOutput")

    F16 = mybir.dt.float16
    ktable = nc.dram_tensor("ktable", [N, 2 * OC], F16)  # internal per-core DRAM

    with tile.TileContext(nc) as tc:
        with (
            tc.tile_pool(name="const", bufs=1) as cp,
            tc.tile_pool(name="work", bufs=3) as wp_,
            tc.tile_pool(name="simp", bufs=2) as sp,
            tc.tile_pool(name="pm_big", bufs=4, space="PSUM") as pmb,
            tc.tile_pool(name="pm_small", bufs=2, space="PSUM") as pms,
            tc.tile_pool(name="dram", bufs=2, space="DRAM") as dp,
        ):
            # ---- load persistent inputs
            xa_t = cp.tile([C + 1, N], F32)
            nc.sync.dma_start(out=xa_t[:], in_=xa[:])
            xr_t = cp.tile([C + 1, ROWS], F32)
            nc.sync.dma_start(out=xr_t[:], in_=xr[:])
            wkv_t = cp.tile([C + 1, 2 * OC], F32)
            nc.sync.dma_start(out=wkv_t[:], in_=wkv[:])
            wq_t = cp.tile([C + 1, OC], F32)
            nc.sync.dma_start(out=wq_t[:], in_=wq[:])
            wpt_t = cp.tile([128, 2, 64], F32)
            nc.sync.dma_start(out=wpt_t[:], in_=wpt[:])
            w1t_t = cp.tile([64, 64], F32)
            nc.sync.dma_start(out=w1t_t[:], in_=w1t[:])
            w2t_t = cp.tile([64, 64], F32)
            nc.sync.dma_start(out=w2t_t[:], in_=w2t[:])
            b1ff_t = cp.tile([64, 1], F32)
            nc.sync.dma_start(out=b1ff_t[:], in_=b1ff[:])
            b2f_t = cp.tile([64, 1], F32)
            nc.sync.dma_start(out=b2f_t[:], in_=b2f[:])
            bo1_t = cp.tile([128, 2], F32)
            nc.sync.dma_start(out=bo1_t[:], in_=bo1[:])
            bo2_t = cp.tile([2, 128], F32)
            nc.sync.dma_start(out=bo2_t[:], in_=bo2[:])
            ones64_t = cp.tile([64, 1], F32)
            nc.sync.dma_start(out=ones64_t[:], in_=ones64[:])
            one1_64_t = cp.tile([1, 64], F32)
            nc.sync.dma_start(out=one1_64_t[:], in_=one1_64[:])
            ones128_t = cp.tile([128, 1], F32)
            nc.sync.dma_start(out=ones128_t[:], in_=ones128[:])

            # ---- B: column-normalized xn (64, N): xn[:, m] = x[:, m] / ||x[:, m]||
            rn = cp.tile([1, N], F32)
            for c in range(N // 512):
                xsq = wp_.tile([C, 512], F32, tag="xsq")
                nc.scalar.activation(out=xsq[:], in_=xa_t[0:C, c * 512:(c + 1) * 512],
                                     func=AF.Square)
                ps = pms.tile([1, 512], F32, space="PSUM", tag="pms")
                nc.tensor.matmul(out=ps[:], lhsT=ones64_t[:], rhs=xsq[:],
                                 start=True, stop=True)
                # rn <- sqrt(sumsq) (then reciprocal in place below)
                nc.scalar.activation(out=rn[:, c * 512:(c + 1) * 512], in_=ps[:], func=AF.Sqrt)
            nc.vector.reciprocal(out=rn[:], in_=rn[:])
            xn = cp.tile([C, N], F32)
            for c in range(N // 512):
                pb = pmb.tile([64, 512], F32, space="PSUM", tag="pm")
                nc.tensor.matmul(out=pb[:], lhsT=one1_64_t[:], rhs=rn[:, c * 512:(c + 1) * 512],
                                 start=True, stop=True)
                nc.vector.tensor_tensor(out=xn[:, c * 512:(c + 1) * 512],
                                        in0=xa_t[0:C, c * 512:(c + 1) * 512],
                                        in1=pb[:], op=ALU.mult)

            # ---- C: k|v table, node-major, k l2-normalized per head
            for t in range(N // 128):
                pkv = pmb.tile([128, 2 * OC], F32, space="PSUM", tag="pm")
                nc.tensor.matmul(out=pkv[:], lhsT=xa_t[:, t * 128:(t + 1) * 128],
                                 rhs=wkv_t[:], start=True, stop=True)
                ksq = wp_.tile([128, NH, INTER], F32, tag="ksq")
                nc.scalar.activation(out=ksq[:], in_=pkv[:, 0:OC].rearrange("p (h f) -> p h f", h=NH),
                                     func=AF.Square)
                rkn = wp_.tile([128, NH], F32, tag="rkn")
                nc.vector.tensor_reduce(out=rkn[:], in_=ksq[:], axis=mybir.AxisListType.X,
                                        op=ALU.add)
                nc.scalar.activation(out=rkn[:], in_=rkn[:], func=AF.Sqrt)
                nc.vector.reciprocal(out=rkn[:], in_=rkn[:])
                rknx = wp_.tile([128, OC], F32, tag="rknx")
                for h in range(NH):
                    nc.scalar.activation(out=rknx[:, h * INTER:(h + 1) * INTER],
                                         in_=rkn[:, h:h + 1].to_broadcast([128, INTER]),
                                         func=AF.Copy)
                tab = wp_.tile([128, 2 * OC], F16, tag="tab")
                nc.vector.tensor_tensor(out=tab[:, 0:OC], in0=pkv[:, 0:OC], in1=rknx[:],
                                        op=ALU.mult)
                nc.scalar.activation(out=tab[:, OC:2 * OC], in_=pkv[:, OC:2 * OC], func=AF.Copy)
                nc.sync.dma_start(out=ktable[t * 128:(t + 1) * 128, :], in_=tab[:])

            # ---- G: q channel-major, l2-normalized per head
            qn = []
            for oh in range(2):
                q_t = cp.tile([128, ROWS], F32, tag=f"q{oh}", name=f"q{oh}")
                for c in range(ROWS // 512):
                    pq = pmb.tile([128, 512], F32, space="PSUM", tag="pm")
                    nc.tensor.matmul(out=pq[:], lhsT=wq_t[:, oh * 128:(oh + 1) * 128],
                                     rhs=xr_t[:, c * 512:(c + 1) * 512], start=True, stop=True)
                    nc.scalar.activation(out=q_t[:, c * 512:(c + 1) * 512], in_=pq[:], func=AF.Copy)
                qn.append(q_t)
            rqs = [cp.tile([2, ROWS], F32, tag=f"rq{oh}", name=f"rq{oh}") for oh in range(2)]
            for oh in range(2):
                for c in range(ROWS // 512):
                    qsq = wp_.tile([128, 512], F32, tag="qsq")
                    nc.scalar.activation(out=qsq[:], in_=qn[oh][:, c * 512:(c + 1) * 512],
                                         func=AF.Square)
                    pn = pms.tile([2, 512], F32, space="PSUM", tag="pms")
                    nc.tensor.matmul(out=pn[:], lhsT=bo1_t[:], rhs=qsq[:],
                                     start=True, stop=True)
                    nc.scalar.activation(out=rqs[oh][:, c * 512:(c + 1) * 512],
                                         in_=pn[:], func=AF.Sqrt)
            for oh in range(2):
                nc.vector.reciprocal(out=rqs[oh][:], in_=rqs[oh][:])
                for c in range(ROWS // 512):
                    pb2 = pmb.tile([128, 512], F32, space="PSUM", tag="pm")
                    nc.tensor.matmul(out=pb2[:], lhsT=bo2_t[:],
                                     rhs=rqs[oh][:, c * 512:(c + 1) * 512],
                                     start=True, stop=True)
                    nc.vector.tensor_tensor(out=qn[oh][:, c * 512:(c + 1) * 512],
                                            in0=qn[oh][:, c * 512:(c + 1) * 512],
                                            in1=pb2[:], op=ALU.mult)

            # ---- D+E interleaved: sim rows -> top5 -> gather -> Laplacian -> product acc
            idx_all = cp.tile([128, 16 * 8], U32)
            acc = cp.tile([128, OC], F32)
            for t in range(ROWS // 128):
                simb = sp.tile([128, N], F32, tag="simb")
                for c in range(N // 512):
                    psim = pmb.tile([128, 512], F32, space="PSUM", tag="pm")
                    nc.tensor.matmul(out=psim[:], lhsT=xr_t[0:C, t * 128:(t + 1) * 128],
                                     rhs=xn[:, c * 512:(c + 1) * 512], start=True, stop=True)
                    nc.scalar.activation(out=simb[:, c * 512:(c + 1) * 512], in_=psim[:],
                                         func=AF.Copy)
                val8 = wp_.tile([128, 8], F32, tag="val8")
                nc.vector.max(out=val8[:], in_=simb[:])
                nc.vector.max_index(out=idx_all[:, t * 8:t * 8 + 8], in_max=val8[:],
                                    in_values=simb[:])

            for t in range(ROWS // 128):
                gbuf = wp_.tile([128, K, 2 * OC], F16, tag="gbuf")
                for g in range(K):
                    nc.gpsimd.indirect_dma_start(
                        out=gbuf[:, g, :], out_offset=None, in_=ktable[:],
                        in_offset=bass.IndirectOffsetOnAxis(
                            ap=idx_all[:, t * 8 + g:t * 8 + g + 1], axis=0),
                    )
                # sum of 5 gathered + extra self (gather 0 is always self);
                # fp16 pairs -> fp32 partials, then fp32 tree
                s01 = wp_.tile([128, 2 * OC], F32, tag="s01")
                nc.vector.tensor_tensor(
                    out=s01[:], in0=gbuf[:, 0, :], in1=gbuf[:, 1, :], op=ALU.add)
                s23 = wp_.tile([128, 2 * OC], F32, tag="s23")
                nc.vector.tensor_tensor(
                    out=s23[:], in0=gbuf[:, 2, :], in1=gbuf[:, 3, :], op=ALU.add)
                s40 = wp_.tile([128, 2 * OC], F32, tag="s40")
                nc.vector.tensor_tensor(
                    out=s40[:], in0=gbuf[:, 4, :], in1=gbuf[:, 0, :], op=ALU.add)
                nc.vector.tensor_tensor(out=s01[:], in0=s01[:], in1=s23[:], op=ALU.add)
                nc.vector.tensor_tensor(out=s01[:], in0=s01[:], in1=s40[:], op=ALU.add)
                # product k*v and accumulate
                prod = wp_.tile([128, OC], F32, tag="prod")
                nc.vector.tensor_tensor(out=prod[:], in0=s01[:, 0:OC], in1=s01[:, OC:2 * OC],
                                        op=ALU.mult)
                if t == 0:
                    nc.vector.tensor_copy(out=acc[:], in_=prod[:])
                else:
                    nc.vector.tensor_tensor(out=acc[:], in0=acc[:], in1=prod[:], op=ALU.add)

            # ---- F: kv partial -> AllReduce over the batch pair -> fold into wp
            kvs = cp.tile([128, 2], F32)
            for m in range(2):
                pr = pms.tile([128, 1], F32, space="PSUM", tag="pms")
                nc.tensor.matmul(out=pr[:], lhsT=acc[:, m * 128:(m + 1) * 128],
                                 rhs=ones128_t[:], start=True, stop=True)
                nc.scalar.activation(out=kvs[:, m:m + 1], in_=pr[:], func=AF.Copy,
                                     scale=1.0 / 36.0)
            kv_in = dp.tile([128, 2], F32)
            kv_out = dp.tile([128, 2], F32)
            nc.sync.dma_start(out=kv_in[:], in_=kvs[:])
            if collective:
                nc.gpsimd.collective_compute(
                    "AllReduce", ALU.add,
                    replica_groups=[[0, 1], [2, 3], [4, 5], [6, 7]],
                    ins=[kv_in[:].opt()], outs=[kv_out[:].opt()],
                )
            else:
                nc.sync.dma_start(out=kv_out[:], in_=kv_in[:])
            kvr = cp.tile([128, 2], F32)
            nc.sync.dma_start(out=kvr[:], in_=kv_out[:])

            wpk = cp.tile([128, 2, 64], F32)
            for m in range(2):
                nc.vector.tensor_scalar_mul(out=wpk[:, m, :], in0=wpt_t[:, m, :],
                                            scalar1=kvr[:, m:m + 1])

            # ---- H: final conv chain on own 2048 columns
            for c in range(ROWS // 512):
                pp1 = pmb.tile([64, 512], F32, space="PSUM", tag="pm")
                nc.tensor.matmul(out=pp1[:], lhsT=wpk[:, 0, :],
                                 rhs=qn[0][:, c * 512:(c + 1) * 512], start=True, stop=False)
                nc.tensor.matmul(out=pp1[:], lhsT=wpk[:, 1, :],
                                 rhs=qn[1][:, c * 512:(c + 1) * 512], start=False, stop=True)
                p1s = wp_.tile([64, 512], F32, tag="p1s")
                nc.scalar.activation(out=p1s[:], in_=pp1[:], func=AF.Copy)
                pp2 = pmb.tile([64, 512], F32, space="PSUM", tag="pm")
                nc.tensor.matmul(out=pp2[:], lhsT=w1t_t[:], rhs=p1s[:], start=True, stop=True)
                p2s = wp_.tile([64, 512], F32, tag="p2s")
                nc.scalar.activation(out=p2s[:], in_=pp2[:], func=AF.Relu, bias=b1ff_t[:, 0:1])
                pp3 = pmb.tile([64, 512], F32, space="PSUM", tag="pm")
                nc.tensor.matmul(out=pp3[:], lhsT=w2t_t[:], rhs=p2s[:], start=True, stop=True)
                outs = wp_.tile([64, 512], F32, tag="outs")
                nc.scalar.activation(out=outs[:], in_=pp3[:], func=AF.Relu, bias=b2f_t[:, 0:1])
                nc.sync.dma_start(out=out_half[:, c * 512:(c + 1) * 512], in_=outs[:])

    nc.compile()
    return nc


def _prep_inputs(inputs):
    f = lambda k: np.asarray(inputs[k], dtype=np.float32)
    x = f('x')
    wk, bk = f('wk'), f('bk')
    wq_, bq = f('wq'), f('bq')
    wv, bv = f('wv'), f('bv')
    wp, bp = f('wp'), f('bp')
    w1, b1 = f('w1'), f('b1')
    w2, b2 = f('w2'), f('b2')
    g1, beta1, m1, v1 = f('g1'), f('beta1'), f('m1'), f('v1')
    g2, beta2, m2, v2 = f('g2'), f('beta2'), f('m2'), f('v2')

    s1 = g1 / np.sqrt(v1 + BN_EPS)
    w1f = s1[:, None] * w1
    b1f = s1 * (b1 - m1) + beta1
    s2 = g2 / np.sqrt(v2 + BN_EPS)
    w2f = s2[:, None] * w2
    b2f_v = s2 * (b2 - m2) + beta2
    b1ff = w1f @ bp + b1f  # bp folded through w1f

    wkv = np.zeros((C + 1, 2 * OC), np.float32)
    wkv[0:C, 0:OC] = wk.T
    wkv[C, 0:OC] = bk
    wkv[0:C, OC:] = wv.T
    wkv[C, OC:] = bv
    wq_a = np.zeros((C + 1, OC), np.float32)
    wq_a[0:C] = wq_.T
    wq_a[C] = bq
    wpt = np.ascontiguousarray(wp.T.reshape(2, 128, 64).transpose(1, 0, 2))

    bo1 = np.zeros((128, 2), np.float32)
    bo1[0:64, 0] = 1.0
    bo1[64:128, 1] = 1.0
    bo2 = np.ascontiguousarray(bo1.T)

    shared = {
        "wkv": wkv, "wq": wq_a, "wpt": wpt,
        "w1t": np.ascontiguousarray(w1f.T), "w2t": np.ascontiguousarray(w2f.T),
        "b1ff": b1ff.reshape(64, 1), "b2f": b2f_v.reshape(64, 1),
        "bo1": bo1, "bo2": bo2,
        "ones64": np.ones((64, 1), np.float32),
        "one1_64": np.ones((1, 64), np.float32),
        "ones128": np.ones((128, 1), np.float32),
    }
    in_maps = []
    for core in range(8):
        b = core // 2
        roff = (core % 2) * ROWS
        xa = np.ones((C + 1, N), np.float32)
        xa[0:C] = x[b].reshape(C, N)
        m = dict(shared)
        m["xa"] = xa
        m["xr"] = np.ascontiguousarray(xa[:, roff:roff + ROWS])
        in_maps.append(m)
    return in_maps


def kernel(**inputs):
    if "nc" not in _CACHE:
        _CACHE["nc"] = _build_bass()
    nc = _CACHE["nc"]
    in_maps = _prep_inputs(inputs)
    res = run_bass_kernel_spmd(nc, in_maps, list(range(8)))
    out = np.empty((B, 64, N), np.float32)
    for core in range(8):
        b = core // 2
        roff = (core % 2) * ROWS
        out[b][:, roff:roff + ROWS] = res.results[core]["out_half"]
    return out.reshape(B, 64, H, W)


if __name__ == "__main__":
    import reference as R
    inputs = R.setup_inputs()
    import os
    os.environ.setdefault("JAX_PLATFORMS", "cpu")
    expected = np.asarray(R.reference(**inputs))
    actual = kernel(**{k: np.asarray(v) for k, v in inputs.items()})
    rel = np.linalg.norm(actual - expected) / np.linalg.norm(expected)
    print("Relative error:", rel)



# revision 7
# speedup vs baseline: 1.4802x; 1.4802x over previous
"""HSpatialHyperGCN Trainium2 kernel (v2).

Shapes (hardcoded): x (4, 64, 64, 64); N = 4096 nodes per batch; 4 heads x 64
inter channels; top-5 cosine-similarity hypergraph; uniform degree 6 Laplacian;
hydra attention (global kv); 1x1-conv + folded-BN chain.

Sharding: 8 cores = 4 batches x 2 node-halves (core = 2*b + half).

v2 design vs v1 baseline (627 us):
  - all big matmuls in float32r (1 cycle/row vs 4 for fp32, ~1.5e-4 rel err)
  - sim rows scanned with MAX8/FIND_INDEX8 *directly on PSUM* in two
    2048-column halves (no PSUM->SBUF copy of the 8.4M-entry sim matrix)
  - halves merged with a bit-pack trick: (value & 0xFFFFF000) | column_index,
    max8 over the 16 packed candidates, AND-decode -> exact-ish top-5 with
    index tie-break, no duplicate-index pathology
  - pack/decode on gpsimd (feeds its own indirect gathers), scan on DVE,
    table normalization on scalar, conv chains on PE: engines balanced so the
    irreducible DVE scan (~140 us) is the critical path
  - k*v Laplacian product accumulated on DVE in fp16; kv reduced via a
    (1/36)-weighted ones-matmul; AllReduce over batch pairs; kv folded into wp
  - lhsT of sim = raw x rows (row scale cannot change a row's top-k)
"""

import sys

sys.path.insert(0, "/opt/trn_rl_repo")

import numpy as np

from concourse import bass, mybir, tile, bacc
from concourse.bass_utils import run_bass_kernel_spmd

F32 = mybir.dt.float32
F32R = mybir.dt.float32r
F16 = mybir.dt.float16
U32 = mybir.dt.uint32
AF = mybir.ActivationFunctionType
ALU = mybir.AluOpType
AXX = mybir.AxisListType.X

B, C, H, W = 4, 64, 64, 64
N = H * W            # 4096
NH = 4
INTER = 64
OC = NH * INTER      # 256
K = 5
ROWS = N // 2        # 2048 rows per core
NT = ROWS // 128     # 16 row tiles per core
BN_EPS = 1e-5

_CACHE = {}


def _build_bass(collective=True):
    nc = bacc.Bacc(None, target_bir_lowering=False, debug=False, num_devices=8)

    xa = nc.dram_tensor("xa", [C + 1, N], F32R, kind="ExternalInput")
    xr = nc.dram_tensor("xr", [C + 1, ROWS], F32R, kind="ExternalInput")
    wkvq = nc.dram_tensor("wkvq", [C + 1, 3 * OC], F32R, kind="ExternalInput")
    wpt = nc.dram_tensor("wpt", [128, 2, 64], F32, kind="ExternalInput")
    w1t = nc.dram_tensor("w1t", [64, 64], F16, kind="ExternalInput")
    w2t = nc.dram_tensor("w2t", [64, 64], F16, kind="ExternalInput")
    b1ff = nc.dram_tensor("b1ff", [64, 1], F32, kind="ExternalInput")
    b2f = nc.dram_tensor("b2f", [64, 1], F32, kind="ExternalInput")
    one64 = nc.dram_tensor("one64", [1, 64], F32R, kind="ExternalInput")
    i64r = nc.dram_tensor("i64r", [64, 64], F32R, kind="ExternalInput")
    i128 = nc.dram_tensor("i128", [128, 128], F32, kind="ExternalInput")
    c36 = nc.dram_tensor("c36", [128, 1], F32, kind="ExternalInput")

    out_half = nc.dram_tensor("out_half", [64, ROWS], F32, kind="ExternalOutput")

    ktable = nc.dram_tensor("ktable", [N, 2 * OC], F16)  # internal per-core DRAM

    with tile.TileContext(nc) as tc:
        with (
            tc.tile_pool(name="const", bufs=1) as cp,
            tc.tile_pool(name="work", bufs=3) as wp_,
            tc.tile_pool(name="tabp", bufs=3) as tabp,
            tc.tile_pool(name="gp", bufs=3) as gp,
            tc.tile_pool(name="ep", bufs=2) as ep,
            tc.tile_pool(name="dram", bufs=2, space="DRAM") as dp,
        ):
            # ---- persistent loads
            xa_t = cp.tile([C + 1, N], F32R)
            nc.sync.dma_start(out=xa_t[:], in_=xa[:])
            xr_t = cp.tile([C + 1, ROWS], F32R)
            nc.sync.dma_start(out=xr_t[:], in_=xr[:])
            wkvq_t = cp.tile([C + 1, 3 * OC], F32R)
            nc.sync.dma_start(out=wkvq_t[:], in_=wkvq[:])
            wpt_t = cp.tile([128, 2, 64], F32)
            nc.sync.dma_start(out=wpt_t[:], in_=wpt[:])
            w1t_t = cp.tile([64, 64], F16)
            nc.sync.dma_start(out=w1t_t[:], in_=w1t[:])
            w2t_t = cp.tile([64, 64], F16)
            nc.sync.dma_start(out=w2t_t[:], in_=w2t[:])
            b1ff_t = cp.tile([64, 1], F32)
            nc.sync.dma_start(out=b1ff_t[:], in_=b1ff[:])
            b2f_t = cp.tile([64, 1], F32)
            nc.sync.dma_start(out=b2f_t[:], in_=b2f[:])
            one64_t = cp.tile([1, 64], F32R)
            nc.sync.dma_start(out=one64_t[:], in_=one64[:])
            i64r_t = cp.tile([64, 64], F32R)
            nc.sync.dma_start(out=i64r_t[:], in_=i64r[:])
            i128_t = cp.tile([128, 128], F32)
            nc.sync.dma_start(out=i128_t[:], in_=i128[:])
            c36_t = cp.tile([128, 1], F32)
            nc.sync.dma_start(out=c36_t[:], in_=c36[:])

            xn = cp.tile([C, N], F32R)
            xss = cp.tile([128, 32], F32)
            rsb = cp.tile([32, 128], F32R)
            rrow = cp.tile([1, N], F32R)
            qn_own = cp.tile([128, NT, OC], F16)
            qc0 = cp.tile([128, ROWS], F16)
            qc1 = cp.tile([128, ROWS], F16)
            idx_all = cp.tile([128, NT * 8], U32)
            acc = cp.tile([128, OC], F32)
            maskv = cp.tile([128, 1], U32)
            nc.vector.memset(maskv[:], 0xFFFFF000)

            # ================= head: B (col norms+xn), C (k|v table), Q =====
            with (
                tc.tile_pool(name="pmt", bufs=2, space="PSUM") as pmt,
                tc.tile_pool(name="pmb", bufs=2, space="PSUM") as pmh,
            ):
                # B1: per-column sumsq via PE transpose + scalar Square accum
                for t in range(N // 128):
                    ptp = pmt.tile([128, 64], F32R, space="PSUM", tag="tp")
                    nc.tensor.transpose(out=ptp[:], in_=xa_t[0:C, t * 128:(t + 1) * 128],
                                        identity=i64r_t[:])
                    xsc = wp_.tile([128, 64], F32, tag="xsc")
                    nc.scalar.activation(out=xsc[:], in_=ptp[:], func=AF.Square,
                                         accum_out=xss[:, t:t + 1])
                # B2: 1/sqrt -> transpose back -> row layout
                nc.scalar.activation(out=xss[:], in_=xss[:], func=AF.Sqrt)
                nc.vector.reciprocal(out=xss[:], in_=xss[:])
                ptr = pmt.tile([32, 128], F32, space="PSUM", tag="tp2")
                nc.tensor.transpose(out=ptr[:], in_=xss[:], identity=i128_t[:])
                nc.scalar.activation(out=rsb[:], in_=ptr[:], func=AF.Copy)
                nc.sync.dma_start(out=rrow[:].rearrange("o (t p) -> o t p", t=32),
                                  in_=rsb[:])
                # B3: xn = xa * bcast(rrow)
                for c in range(N // 512):
                    pb = pmh.tile([C, 512], F32, space="PSUM", tag="pb")
                    nc.tensor.matmul(out=pb[:], lhsT=one64_t[:],
                                     rhs=rrow[:, c * 512:(c + 1) * 512],
                                     start=True, stop=True)
                    nc.vector.tensor_tensor(out=xn[:, c * 512:(c + 1) * 512],
                                            in0=xa_t[0:C, c * 512:(c + 1) * 512].bitcast(F32),
                                            in1=pb[:], op=ALU.mult)

            # C: k|v table + Q (separate PSUM scope)
            with tc.tile_pool(name="pmc", bufs=2, space="PSUM") as pmh:
                for t in range(N // 128):
                    pkv = pmh.tile([128, 2 * OC], F32, space="PSUM", tag="pkv")
                    nc.tensor.matmul(out=pkv[:], lhsT=xa_t[:, t * 128:(t + 1) * 128],
                                     rhs=wkvq_t[:, 0:2 * OC], start=True, stop=True)
                    ksq = wp_.tile([128, OC], F32, tag="ksq")
                    nc.scalar.activation(out=ksq[:], in_=pkv[:, 0:OC], func=AF.Square)
                    rkn = wp_.tile([128, NH], F32, tag="rkn")
                    nc.vector.tensor_reduce(
                        out=rkn[:], in_=ksq[:].rearrange("p (h f) -> p h f", h=NH),
                        axis=AXX, op=ALU.add)
                    nc.scalar.activation(out=rkn[:], in_=rkn[:], func=AF.Sqrt)
                    nc.vector.reciprocal(out=rkn[:], in_=rkn[:])
                    tab = tabp.tile([128, 2 * OC], F16, tag="tab")
                    for h in range(NH):
                        nc.scalar.activation(out=tab[:, h * 64:(h + 1) * 64],
                                             in_=pkv[:, h * 64:(h + 1) * 64],
                                             func=AF.Copy, scale=rkn[:, h:h + 1])
                    nc.scalar.activation(out=tab[:, OC:2 * OC], in_=pkv[:, OC:2 * OC],
                                         func=AF.Copy)
                    nc.sync.dma_start(out=ktable[t * 128:(t + 1) * 128, :], in_=tab[:])

                # Q head: q = Wq xr (+bq), node-major fp16 (normalized later)
                for t in range(NT):
                    pq = pmh.tile([128, OC], F32, space="PSUM", tag="pq")
                    nc.tensor.matmul(out=pq[:], lhsT=xr_t[:, t * 128:(t + 1) * 128],
                                     rhs=wkvq_t[:, 2 * OC:3 * OC], start=True, stop=True)
                    nc.scalar.activation(out=qn_own[:, t, :], in_=pq[:], func=AF.Copy)

            # ================= D/E: sim -> top5 -> gather -> laplacian product
            with tc.tile_pool(name="pms", bufs=2, space="PSUM") as pms:

                def emit_d(t):
                    v8 = wp_.tile([128, 16], F32, tag="v8")
                    i8 = wp_.tile([128, 16], U32, tag="i8")
                    packed = wp_.tile([128, 16], U32, tag="pk")
                    for half in range(2):
                        sim = pms.tile([128, 2048], F32, space="PSUM", tag="sim")
                        for c in range(4):
                            cc = half * 4 + c
                            nc.tensor.matmul(out=sim[:, c * 512:(c + 1) * 512],
                                             lhsT=xr_t[0:C, t * 128:(t + 1) * 128],
                                             rhs=xn[:, cc * 512:(cc + 1) * 512],
                                             start=True, stop=True)
                        sl = slice(half * 8, half * 8 + 8)
                        nc.vector.max(out=v8[:, sl], in_=sim[:])
                        nc.vector.max_index(out=i8[:, sl], in_max=v8[:, sl],
                                            in_values=sim[:])
                        if half == 1:
                            nc.vector.tensor_scalar(out=i8[:, sl], in0=i8[:, sl],
                                                    scalar1=0x800, scalar2=None,
                                                    op0=ALU.bitwise_or)
                        nc.vector.scalar_tensor_tensor(
                            out=packed[:, sl], in0=v8[:, sl].bitcast(U32),
                            scalar=maskv[:, 0:1], in1=i8[:, sl],
                            op0=ALU.bitwise_and, op1=ALU.bitwise_or)
                    gpk = wp_.tile([128, 8], F32, tag="gpk")
                    nc.vector.max(out=gpk[:], in_=packed[:].bitcast(F32))
                    nc.vector.tensor_scalar(out=idx_all[:, t * 8:t * 8 + 8],
                                            in0=gpk[:].bitcast(U32), scalar1=0xFFF,
                                            scalar2=None, op0=ALU.bitwise_and)
                    gbuf = gp.tile([128, K, 2 * OC], F16, tag="gbuf")
                    for g in range(K):
                        nc.gpsimd.indirect_dma_start(
                            out=gbuf[:, g, :], out_offset=None, in_=ktable[:],
                            in_offset=bass.IndirectOffsetOnAxis(
                                ap=idx_all[:, t * 8 + g:t * 8 + g + 1], axis=0),
                        )
                    return gbuf

                def emit_e(t, gbuf):
                    # S = 2*g0 + g1 + g2 + g3 + g4 (self-loop: g0 is always self)
                    s1 = ep.tile([128, 2 * OC], F16, tag="s1")
                    nc.vector.scalar_tensor_tensor(
                        out=s1[:], in0=gbuf[:, 0, :], scalar=2.0, in1=gbuf[:, 1, :],
                        op0=ALU.mult, op1=ALU.add)
                    s2 = ep.tile([128, 2 * OC], F16, tag="s2")
                    nc.vector.scalar_tensor_tensor(
                        out=s2[:], in0=gbuf[:, 2, :], scalar=1.0, in1=gbuf[:, 3, :],
                        op0=ALU.mult, op1=ALU.add)
                    nc.vector.scalar_tensor_tensor(
                        out=s1[:], in0=s2[:], scalar=1.0, in1=s1[:],
                        op0=ALU.mult, op1=ALU.add)
                    nc.vector.scalar_tensor_tensor(
                        out=s1[:], in0=gbuf[:, 4, :], scalar=1.0, in1=s1[:],
                        op0=ALU.mult, op1=ALU.add)
                    prod = ep.tile([128, OC], F16, tag="prod")
                    nc.vector.scalar_tensor_tensor(
                        out=prod[:], in0=s1[:, 0:OC], scalar=1.0, in1=s1[:, OC:2 * OC],
                        op0=ALU.mult, op1=ALU.mult)
                    if t == 0:
                        nc.vector.tensor_copy(out=acc[:], in_=prod[:])
                    else:
                        nc.vector.tensor_tensor(out=acc[:], in0=acc[:], in1=prod[:],
                                                op=ALU.add)

                def emit_q(t):
                    qsq = wp_.tile([128, OC], F32, tag="qsq")
                    nc.scalar.activation(out=qsq[:], in_=qn_own[:, t, :], func=AF.Square)
                    rq = wp_.tile([128, NH], F32, tag="rq")
                    nc.vector.tensor_reduce(
                        out=rq[:], in_=qsq[:].rearrange("p (h f) -> p h f", h=NH),
                        axis=AXX, op=ALU.add)
                    nc.scalar.activation(out=rq[:], in_=rq[:], func=AF.Sqrt)
                    nc.vector.reciprocal(out=rq[:], in_=rq[:])
                    for h in range(NH):
                        nc.scalar.activation(out=qn_own[:, t, h * 64:(h + 1) * 64],
                                             in_=qn_own[:, t, h * 64:(h + 1) * 64],
                                             func=AF.Copy, scale=rq[:, h:h + 1])
                    nc.sync.dma_start_transpose(out=qc0[:, t * 128:(t + 1) * 128],
                                                in_=qn_own[:, t, 0:128])
                    nc.sync.dma_start_transpose(out=qc1[:, t * 128:(t + 1) * 128],
                                                in_=qn_own[:, t, 128:256])

                gbufs = {}
                for t in range(NT):
                    gbufs[t] = emit_d(t)
                    emit_q(t)
                    if t >= 2:
                        emit_e(t - 2, gbufs.pop(t - 2))
                emit_e(NT - 2, gbufs.pop(NT - 2))
                emit_e(NT - 1, gbufs.pop(NT - 1))

            # ================= F: kv reduce + AllReduce + fold; H: conv chain
            with tc.tile_pool(name="pmz", bufs=2, space="PSUM") as pmz:
                pkvs = pmz.tile([1, OC], F32, space="PSUM", tag="kv")
                nc.tensor.matmul(out=pkvs[:], lhsT=c36_t[:], rhs=acc[:],
                                 start=True, stop=True)
                kvs = wp_.tile([1, OC], F32, tag="kvs")
                nc.scalar.activation(out=kvs[:], in_=pkvs[:], func=AF.Copy)
                kv_in = dp.tile([1, OC], F32)
                kv_out = dp.tile([1, OC], F32)
                nc.sync.dma_start(out=kv_in[:], in_=kvs[:])
                if collective:
                    nc.gpsimd.collective_compute(
                        "AllReduce", ALU.add,
                        replica_groups=[[0, 1], [2, 3], [4, 5], [6, 7]],
                        ins=[kv_in[:].opt()], outs=[kv_out[:].opt()],
                    )
                else:
                    nc.sync.dma_start(out=kv_out[:], in_=kv_in[:])
                kvr = cp.tile([128, 2], F32)
                nc.sync.dma_start(out=kvr[:],
                                  in_=kv_out[:].rearrange("o (m p) -> o p m", m=2))

                wpk = cp.tile([128, 2, 64], F16)
                for m in range(2):
                    nc.vector.tensor_scalar_mul(out=wpk[:, m, :],
                                                in0=wpt_t[:, m, :],
                                                scalar1=kvr[:, m:m + 1])

                qcs = [qc0, qc1]
                for c in range(ROWS // 512):
                    cs = slice(c * 512, (c + 1) * 512)
                    pp1 = pmz.tile([64, 512], F32, space="PSUM", tag="pp")
                    for m in range(2):
                        nc.tensor.matmul(out=pp1[:], lhsT=wpk[:, m, :],
                                         rhs=qcs[m][:, cs],
                                         start=(m == 0), stop=(m == 1))
                    p1s = wp_.tile([64, 512], F16, tag="p1s")
                    nc.scalar.activation(out=p1s[:], in_=pp1[:], func=AF.Copy)
                    pp2 = pmz.tile([64, 512], F32, space="PSUM", tag="pp")
                    nc.tensor.matmul(out=pp2[:], lhsT=w1t_t[:], rhs=p1s[:],
                                     start=True, stop=True)
                    p2s = wp_.tile([64, 512], F16, tag="p2s")
                    nc.scalar.activation(out=p2s[:], in_=pp2[:], func=AF.Relu,
                                         bias=b1ff_t[:, 0:1])
                    pp3 = pmz.tile([64, 512], F32, space="PSUM", tag="pp")
                    nc.tensor.matmul(out=pp3[:], lhsT=w2t_t[:], rhs=p2s[:],
                                     start=True, stop=True)
                    outs = wp_.tile([64, 512], F32, tag="outs")
                    nc.scalar.activation(out=outs[:], in_=pp3[:], func=AF.Relu,
                                         bias=b2f_t[:, 0:1])
                    nc.sync.dma_start(out=out_half[:, cs], in_=outs[:])

    nc.compile()
    return nc


def _prep_inputs(inputs):
    f = lambda k: np.asarray(inputs[k], dtype=np.float32)
    x = f('x')
    wk, bk = f('wk'), f('bk')
    wq_, bq = f('wq'), f('bq')
    wv, bv = f('wv'), f('bv')
    wp, bp = f('wp'), f('bp')
    w1, b1 = f('w1'), f('b1')
    w2, b2 = f('w2'), f('b2')
    g1, beta1, m1, v1 = f('g1'), f('beta1'), f('m1'), f('v1')
    g2, beta2, m2, v2 = f('g2'), f('beta2'), f('m2'), f('v2')

    s1 = g1 / np.sqrt(v1 + BN_EPS)
    w1f = s1[:, None] * w1
    b1f = s1 * (b1 - m1) + beta1
    s2 = g2 / np.sqrt(v2 + BN_EPS)
    w2f = s2[:, None] * w2
    b2f_v = s2 * (b2 - m2) + beta2
    b1ff = w1f @ bp + b1f  # bp folded through w1f

    wkvq = np.zeros((C + 1, 3 * OC), np.float32)
    wkvq[0:C, 0:OC] = wk.T
    wkvq[C, 0:OC] = bk
    wkvq[0:C, OC:2 * OC] = wv.T
    wkvq[C, OC:2 * OC] = bv
    wkvq[0:C, 2 * OC:] = wq_.T
    wkvq[C, 2 * OC:] = bq
    wpt = np.ascontiguousarray(wp.T.reshape(2, 128, 64).transpose(1, 0, 2))

    shared = {
        "wkvq": wkvq, "wpt": wpt,
        "w1t": np.ascontiguousarray(w1f.T).astype(np.float16),
        "w2t": np.ascontiguousarray(w2f.T).astype(np.float16),
        "b1ff": b1ff.reshape(64, 1), "b2f": b2f_v.reshape(64, 1),
        "one64": np.ones((1, 64), np.float32),
        "i64r": np.eye(64, dtype=np.float32),
        "i128": np.eye(128, dtype=np.float32),
        "c36": np.full((128, 1), 1.0 / 36.0, np.float32),
    }
    in_maps = []
    for core in range(8):
        b = core // 2
        roff = (core % 2) * ROWS
        xa = np.ones((C + 1, N), np.float32)
        xa[0:C] = x[b].reshape(C, N)
        m = dict(shared)
        m["xa"] = xa
        m["xr"] = np.ascontiguousarray(xa[:, roff:roff + ROWS])
        in_maps.append(m)
    return in_maps


def kernel(**inputs):
    if "nc" not in _CACHE:
        _CACHE["nc"] = _build_bass()
    nc = _CACHE["nc"]
    in_maps = _prep_inputs(inputs)
    res = run_bass_kernel_spmd(nc, in_maps, list(range(8)))
    out = np.empty((B, 64, N), np.float32)
    for core in range(8):
        b = core // 2
        roff = (core % 2) * ROWS
        out[b][:, roff:roff + ROWS] = res.results[core]["out_half"]
    return out.reshape(B, 64, H, W)


if __name__ == "__main__":
    import os
    os.environ.setdefault("JAX_PLATFORMS", "cpu")
    import reference as R
    inputs = R.setup_inputs()
    expected = np.asarray(R.reference(**inputs))
    actual = kernel(**{k: np.asarray(v) for k, v in inputs.items()})
    rel = np.linalg.norm(actual - expected) / np.linalg.norm(expected)
    print("Relative error:", rel)
